# revision 37
# baseline (speedup 1.0000x reference)
"""Trainium2 Bass kernel for nn_Attention_46995532153449.

Module: qkv = x @ w_qkv; per-head scores = q k^T * hd^-0.5; softmax over the
HEAD axis (axis=1); attn = probs @ v; out = attn @ w_proj + b_proj.

Shapes: B=2, T=2048, D=1024, H=16, HD=64.

Sharding: data-parallel over (batch, query-block): core c handles batch c//4
and queries [(c%4)*512, (c%4+1)*512). The head-axis softmax is local (each
core holds all 16 heads for its query slice). K/V for the whole batch are
recomputed per core (collectives are priced far above their compute saving
by the cost model, so no cross-core exchange).

Structure (all chosen against the TimelineSim cost model):
  - host feeds x^T fp16 with columns ROTATED so the core's own 512 queries
    are columns 0:512 (one SPMD program, per-core data). Key order is a
    rotation, which attention is invariant to.
  - attention runs as two passes over the 16 key chunks (qh = 256-query
    halves) to fit PSUM. Pass A also produces K/V, software-pipelined as
    per-chunk lookahead filler (2 k^T tiles + 1 v tile per chunk) emitted
    BETWEEN a chunk's scores and its PV so the PE never stalls on the
    softmax chain; PV lags one chunk.
  - PV uses the attn[q, d] orientation: lhsT = P^T tile [keys, q] (M=128),
    rhs = v [keys, 64] (N=64) -> full 128-partition output at half the PE
    cycles of the attn^T[d, q] orientation. Accumulation runs over all 16
    key chunks directly in PSUM (pre-zeroed banks + start=False: column-
    split accumulation groups must not use start=True, which clears the
    whole partition row of a bank on this HW). No spill-adds.
  - attn[q, d] -> attn^T[d, q] via dma_start_transpose (xbar): DMA cycles
    only, no PE/ACT/DVE time, no PSUM.
  - softmax: exp on ScalarE (fused scale + PSUM evacuation, [128,1024]
    pieces, double-buffered so ACT pipelines with the scores matmuls);
    head-sum tree + reciprocal on VectorE; P = E*R split VectorE/GpSimd.
  - the output projection runs at the tail (PSUM is fully booked during
    pass B); its latency chain is short.
"""

import numpy as np

import concourse.bacc as bacc
import concourse.mybir as mybir
import concourse.tile as tile
from concourse import bass_utils

B, T, D, H = 2, 2048, 1024, 16
HD = D // H          # 64
SCALE = HD ** -0.5   # 0.125
NCORES = 8
QS = B * T // NCORES  # 512 queries per core
QH = QS // 2          # 256-query halves (PSUM budget)
DC = D // 128         # 8 d/e chunks of 128
TC = T // 128         # 16 key chunks of 128

F16 = mybir.dt.float16
F32 = mybir.dt.float32
ADD = mybir.AluOpType.add
MULT = mybir.AluOpType.mult
EXP = mybir.ActivationFunctionType.Exp

_CACHED_NC = None


def _build_nc():
    nc = bacc.Bacc(
        "TRN2", target_bir_lowering=False, debug=False, enable_asserts=False
    )

    xt_d = nc.dram_tensor("xt", [D, T], F16, kind="ExternalInput").ap()
    wq_d = nc.dram_tensor("wq", [D, D], F16, kind="ExternalInput").ap()
    wk_d = nc.dram_tensor("wk", [D, D], F16, kind="ExternalInput").ap()
    wv_d = nc.dram_tensor("wv", [D, D], F16, kind="ExternalInput").ap()
    wp_d = nc.dram_tensor("wp", [D, D], F16, kind="ExternalInput").ap()
    bias_d = nc.dram_tensor("bias", [128, D], F32, kind="ExternalInput").ap()
    out_d = nc.dram_tensor("out", [QS, D], F32, kind="ExternalOutput").ap()

    def chunked(ap):  # [(c p), f] -> [p, c, f]
        return ap.rearrange("(c p) f -> p c f", p=128)

    xt_ch = chunked(xt_d)
    wq_ch = chunked(wq_d)
    out_ch = chunked(out_d)

    with tile.TileContext(nc) as tc:
        with tc.tile_pool(name="persist", bufs=1) as pp:
            kT = pp.tile([128, DC, T], F16)      # k^T: [e, t], e-chunk major
            v_sb = pp.tile([128, TC, D], F16)    # v: [t, e], t-chunk major
            # zero-padded q^T: for head pair pr and query half qh, columns
            # [0:QH] hold head 2pr's q^T at partitions 0:64 (zeros below),
            # columns [QH:2QH] hold head 2pr+1's at partitions 64:128, so
            # every scores matmul is a full-128-partition K=128 matmul.
            qpad = pp.tile([128, DC, 2, 2 * QH], F16)
            att = pp.tile([128, 2, D], F16)      # attn [q, d], per-qh reuse
            aT = pp.tile([128, DC, QS], F16)     # attn^T [d, q]
            wp_sb = pp.tile([128, DC, D], F16)
            bi_sb = pp.tile([128, D], F32)

            nc.gpsimd.memset(qpad, 0.0)

            with tc.tile_pool(name="pA", bufs=1) as pA:
                xT = pA.tile([128, DC, T], F16)
                wk_sb = pA.tile([128, DC, D], F16)
                wv_sb = pA.tile([128, DC, D], F16)

                with (
                    tc.tile_pool(name="qpool", bufs=1) as qp,
                    tc.tile_pool(name="qpsum", bufs=2, space="PSUM") as qpsum,
                ):
                    wq_sb = qp.tile([128, DC, D], F16)
                    # DMA ring order: first half of wq -> own x^T piece ->
                    # rest of wq -> wk -> remaining x^T pieces -> wv -> wp
                    # -> bias. Q gates on the first three; K production on
                    # wk + successive x^T pieces; wp/bias only matter at
                    # the tail.
                    nc.sync.dma_start(wq_sb[:, :, 0:128], wq_ch[:, :, 0:128])
                    nc.sync.dma_start(xT[:, :, 0:512], xt_ch[:, :, 0:512])
                    nc.sync.dma_start(
                        wq_sb[:, :, 128:1024], wq_ch[:, :, 128:1024]
                    )
                    nc.sync.dma_start(wk_sb, chunked(wk_d))
                    for tj in range(1, 4):
                        nc.sync.dma_start(
                            xT[:, :, tj * 512:(tj + 1) * 512],
                            xt_ch[:, :, tj * 512:(tj + 1) * 512],
                        )
                    nc.sync.dma_start(wv_sb, chunked(wv_d))
                    nc.sync.dma_start(wp_sb, chunked(wp_d))
                    nc.sync.dma_start(bi_sb, bias_d)

                    # q^T[e, q] for this core's queries (x^T cols 0:512),
                    # written into the zero-padded layout.
                    for ej in range(DC):
                        qps = qpsum.tile([128, 512], F32, tag="qps")
                        for jd in range(DC):
                            nc.tensor.matmul(
                                qps,
                                lhsT=wq_sb[:, jd, ej * 128:(ej + 1) * 128],
                                rhs=xT[:, jd, 0:512],
                                start=(jd == 0),
                                stop=(jd == DC - 1),
                            )
                        cp = (
                            nc.scalar.copy if ej % 2 == 0
                            else nc.vector.tensor_copy
                        )
                        for sel in range(2):
                            cp(
                                qpad[0:64, ej, sel, 0:QH],
                                qps[0:64, sel * QH:(sel + 1) * QH],
                            )
                            cp(
                                qpad[64:128, ej, sel, QH:2 * QH],
                                qps[64:128, sel * QH:(sel + 1) * QH],
                            )

                # ---------------- pass A: qh=0 + K/V production ----------
                with (
                    tc.tile_pool(name="accA", bufs=1, space="PSUM") as accp,
                    tc.tile_pool(name="scA", bufs=1, space="PSUM") as scp,
                    tc.tile_pool(name="kvps", bufs=2, space="PSUM") as kvp,
                    tc.tile_pool(name="Ep", bufs=2) as Ep,
                    tc.tile_pool(name="smx", bufs=1) as smx,
                    tc.tile_pool(name="rpool", bufs=2) as rpool,
                ):
                    def emit_k(tj, ej):
                        kps = kvp.tile([128, 512], F32, tag="kv")
                        for jd in range(DC):
                            nc.tensor.matmul(
                                kps,
                                lhsT=wk_sb[:, jd, ej * 128:(ej + 1) * 128],
                                rhs=xT[:, jd, tj * 512:(tj + 1) * 512],
                                start=(jd == 0),
                                stop=(jd == DC - 1),
                            )
                        nc.scalar.copy(
                            kT[:, ej, tj * 512:(tj + 1) * 512], kps
                        )

                    def emit_v(kc):
                        for eh in range(2):
                            vps = kvp.tile([128, 512], F32, tag="kv")
                            for jd in range(DC):
                                nc.tensor.matmul(
                                    vps,
                                    lhsT=xT[:, jd, kc * 128:(kc + 1) * 128],
                                    rhs=wv_sb[:, jd,
                                              eh * 512:(eh + 1) * 512],
                                    start=(jd == 0),
                                    stop=(jd == DC - 1),
                                )
                            cp = (
                                nc.vector.tensor_copy if eh == 0
                                else nc.scalar.copy
                            )
                            cp(v_sb[:, kc, eh * 512:(eh + 1) * 512], vps)

                    acc0 = accp.tile([128, D], F32)
                    acc1 = accp.tile([128, D], F32)
                    accs = [acc0, acc1]
                    # column-split accumulation groups share PSUM banks;
                    # start=True clears beyond its own columns on this HW,
                    # so pre-zero the banks and accumulate with start=False.
                    nc.vector.memset(acc0, 0.0)
                    nc.vector.memset(acc1, 0.0)

                    # prologue: k^T superstep 0 (keys 0:512) + v chunk 0
                    for ej in range(DC):
                        emit_k(0, ej)
                    emit_v(0)

                    pend = []  # softmax+PV closures, lagged one chunk
                    for kc in range(TC):
                        # K/V lookahead fillers, emitted BETWEEN score
                        # groups: the scores PSUM tile is single-buffered
                        # (bank budget), so group g+1's matmuls wait on
                        # group g's exp — the filler keeps the PE busy
                        # through that and through the softmax chain.
                        fillers = []
                        if kc < 12:
                            tj = kc // 4 + 1
                            fillers.append(
                                lambda tj=tj, e=2 * (kc % 4): emit_k(tj, e)
                            )
                            fillers.append(
                                lambda tj=tj, e=2 * (kc % 4) + 1:
                                emit_k(tj, e)
                            )
                        if kc < TC - 1:
                            fillers.append(lambda kc=kc: emit_v(kc + 1))
                        Et = _scores(nc, scp, Ep, kT, qpad, kc, qh=0,
                                     fillers=fillers)
                        pend.append(
                            lambda kc=kc, Et=Et: _softmax_pv(
                                nc, smx, rpool, v_sb, accs, Et, kc,
                                dve_heads=10
                            )
                        )
                        if len(pend) > 1:
                            pend.pop(0)()
                    pend.pop(0)()
                    nc.scalar.copy(att[:, 0, :], acc0)
                    nc.vector.tensor_copy(att[:, 1, :], acc1)

            # attn^T for qh0 via xbar DMA transpose (DMA engine only)
            for qc in range(2):
                nc.sync.dma_start_transpose(
                    aT[:, :, qc * 128:(qc + 1) * 128], att[:, qc, :]
                )

            # ---------------- pass B: qh=1 ----------
            with (
                tc.tile_pool(name="accB", bufs=1, space="PSUM") as accpB,
                tc.tile_pool(name="scB", bufs=2, space="PSUM") as scpB,
                tc.tile_pool(name="EpB", bufs=4) as EpB,
                tc.tile_pool(name="smxB", bufs=1) as smxB,
                tc.tile_pool(name="rpoolB", bufs=2) as rpoolB,
            ):
                # pass B accumulates attn^T [d, q] directly (PV orientation
                # A, 2x the PE cycles of orientation B — free here since
                # pass B is ACT-bound) so the tail needs no evac+transpose.
                accT = accpB.tile([128, DC, QH], F32)
                nc.vector.memset(accT, 0.0)
                pend = []
                for kc in range(TC):
                    Et = _scores(nc, scpB, EpB, kT, qpad, kc, qh=1)
                    pend.append(
                        lambda kc=kc, Et=Et: _softmax_pvT(
                            nc, smxB, rpoolB, v_sb, accT, Et, kc,
                            dve_heads=10
                        )
                    )
                    if len(pend) > 2:
                        pend.pop(0)()
                while pend:
                    pend.pop(0)()
                nc.scalar.copy(aT[:, 0:4, 256:512], accT[:, 0:4, :])
                nc.vector.tensor_copy(aT[:, 4:8, 256:512], accT[:, 4:8, :])

            # ---------------- tail: output projection ----------
            with (
                tc.tile_pool(name="prjps", bufs=2, space="PSUM") as prjp,
                tc.tile_pool(name="outp", bufs=2) as outp,
            ):
                # qh0's projection first: its aT half has been ready since
                # pass A, so it overlaps qh1's evac/transpose chain. Wide
                # [128,1024] tiles: the two e-halves land in separate banks
                # (start=True safe), one bias add + one DMA per block.
                for qs in (0, 1, 2, 3):
                    pm = prjp.tile([128, D], F32, tag="pm")
                    for eh in range(2):
                        for jd in range(DC):
                            nc.tensor.matmul(
                                pm[:, eh * 512:(eh + 1) * 512],
                                lhsT=aT[:, jd, qs * 128:(qs + 1) * 128],
                                rhs=wp_sb[:, jd, eh * 512:(eh + 1) * 512],
                                start=(jd == 0),
                                stop=(jd == DC - 1),
                            )
                    ot = outp.tile([128, D], F32, tag="ot")
                    nc.vector.tensor_tensor(ot, pm, bi_sb, ADD)
                    nc.sync.dma_start(out_ch[:, qs, :], ot)

    nc.compile()
    return nc


def _scores(nc, scp, Ep, kT, qpad, kc, qh, fillers=()):
    """QK^T scores + fused scale/exp evacuation for one key chunk.

    `fillers` are emitted between score groups to give the PE independent
    work while the single-buffered scores tile round-trips through exp.
    """
    fillers = list(fillers)
    Et = Ep.tile([128, H, QH], F16, tag="E")
    for g in range(4):
        sc = scp.tile([128, 1024], F32, tag="sc")
        for i in range(2):
            pr = 2 * g + i
            nc.tensor.matmul(
                sc[:, i * 512:(i + 1) * 512],
                lhsT=kT[:, pr, kc * 128:(kc + 1) * 128],
                rhs=qpad[:, pr, qh, :],
                start=True,
                stop=True,
            )
        nc.scalar.activation(Et[:, 4 * g:4 * g + 4, :], sc, EXP, scale=SCALE)
        if fillers:
            fillers.pop(0)()
    while fillers:
        fillers.pop(0)()
    return Et


def _softmax_pv(nc, smx, rpool, v_sb, accs, Et, kc, dve_heads):
    """Head-axis softmax + PV accumulation for one key chunk."""
    # S = sum over heads (log tree), R = 1/S, P = E * R broadcast
    tmp = smx.tile([128, H // 2, QH], F16, tag="tmp")
    nc.vector.tensor_tensor(tmp, Et[:, 0:8], Et[:, 8:16], ADD)
    nc.vector.tensor_tensor(tmp[:, 0:4], tmp[:, 0:4], tmp[:, 4:8], ADD)
    nc.vector.tensor_tensor(tmp[:, 0:2], tmp[:, 0:2], tmp[:, 2:4], ADD)
    nc.vector.tensor_tensor(tmp[:, 0:1], tmp[:, 0:1], tmp[:, 1:2], ADD)
    r = rpool.tile([128, 1, QH], F16, tag="r")
    with nc.allow_low_precision(
        reason="softmax denominator reciprocal in fp16"
    ):
        nc.vector.reciprocal(r, tmp[:, 0:1])
    a = dve_heads
    nc.vector.tensor_tensor(
        Et[:, 0:a], Et[:, 0:a], r.to_broadcast([128, a, QH]), MULT
    )
    nc.gpsimd.tensor_tensor(
        Et[:, a:H], Et[:, a:H], r.to_broadcast([128, H - a, QH]), MULT
    )
    # PV: attn[q, d] orientation, PSUM accumulation across all key chunks
    for h in range(H):
        for qc in range(2):
            nc.tensor.matmul(
                accs[qc][:, h * HD:(h + 1) * HD],
                lhsT=Et[:, h, qc * 128:(qc + 1) * 128],
                rhs=v_sb[:, kc, h * HD:(h + 1) * HD],
                start=False,
                stop=(kc == TC - 1),
                skip_group_check=True,
            )


def _softmax_pvT(nc, smx, rpool, v_sb, accT, Et, kc, dve_heads):
    """Like _softmax_pv, but PV writes attn^T [d, q] (orientation A):
    lhsT = v tile [keys, 64], rhs = P^T [keys, q]; head pairs share a
    PSUM region via partition halves."""
    tmp = smx.tile([128, H // 2, QH], F16, tag="tmp")
    nc.vector.tensor_tensor(tmp, Et[:, 0:8], Et[:, 8:16], ADD)
    nc.vector.tensor_tensor(tmp[:, 0:4], tmp[:, 0:4], tmp[:, 4:8], ADD)
    nc.vector.tensor_tensor(tmp[:, 0:2], tmp[:, 0:2], tmp[:, 2:4], ADD)
    nc.vector.tensor_tensor(tmp[:, 0:1], tmp[:, 0:1], tmp[:, 1:2], ADD)
    r = rpool.tile([128, 1, QH], F16, tag="r")
    with nc.allow_low_precision(
        reason="softmax denominator reciprocal in fp16"
    ):
        nc.vector.reciprocal(r, tmp[:, 0:1])
    a = dve_heads
    nc.vector.tensor_tensor(
        Et[:, 0:a], Et[:, 0:a], r.to_broadcast([128, a, QH]), MULT
    )
    nc.gpsimd.tensor_tensor(
        Et[:, a:H], Et[:, a:H], r.to_broadcast([128, H - a, QH]), MULT
    )
    for h in range(H):
        dc, half = h // 2, h % 2
        lo = 64 * half
        nc.tensor.matmul(
            accT[lo:lo + 64, dc, :],
            lhsT=v_sb[:, kc, h * HD:(h + 1) * HD],
            rhs=Et[:, h, :],
            start=False,
            stop=(kc == TC - 1),
            skip_group_check=True,
        )


def get_nc():
    global _CACHED_NC
    if _CACHED_NC is None:
        _CACHED_NC = _build_nc()
    return _CACHED_NC


def kernel(x, w_qkv, w_proj, b_proj, _trace=False, _tmpdir=None):
    x = np.asarray(x, dtype=np.float32)
    w_qkv = np.asarray(w_qkv, dtype=np.float32)
    w_proj = np.asarray(w_proj, dtype=np.float32)
    b_proj = np.asarray(b_proj, dtype=np.float32)

    # Host-side layout prep: transpose + fp16 casts + per-core rotation.
    xT = [np.ascontiguousarray(x[b].T).astype(np.float16) for b in range(B)]
    wq = np.ascontiguousarray(w_qkv[:, 0:D]).astype(np.float16)
    wk = np.ascontiguousarray(w_qkv[:, D:2 * D]).astype(np.float16)
    wv = np.ascontiguousarray(w_qkv[:, 2 * D:3 * D]).astype(np.float16)
    wp = w_proj.astype(np.float16)
    bias = np.ascontiguousarray(
        np.broadcast_to(b_proj, (128, D))
    ).astype(np.float32)

    in_maps = []
    for c in range(NCORES):
        b = c // (NCORES // B)
        qofs = (c % (NCORES // B)) * QS
        xt_rot = np.ascontiguousarray(np.roll(xT[b], -qofs, axis=1))
        in_maps.append(
            {
                "xt": xt_rot,
                "wq": wq,
                "wk": wk,
                "wv": wv,
                "wp": wp,
                "bias": bias,
            }
        )

    nc = get_nc()
    res = bass_utils.run_bass_kernel_spmd(
        nc,
        in_maps,
        core_ids=list(range(NCORES)),
        trace=_trace,
        tmpdir=_tmpdir,
    )

    out = np.empty((B, T, D), dtype=np.float32)
    for c in range(NCORES):
        b = c // (NCORES // B)
        qofs = (c % (NCORES // B)) * QS
        out[b, qofs:qofs + QS] = res.results[c]["out"]
    if _trace:
        kernel._last_results = res
    return out


# revision 38
# speedup vs baseline: 1.0065x; 1.0065x over previous
"""Trainium2 Bass kernel for nn_Attention_46995532153449.

Module: qkv = x @ w_qkv; per-head scores = q k^T * hd^-0.5; softmax over the
HEAD axis (axis=1); attn = probs @ v; out = attn @ w_proj + b_proj.

Shapes: B=2, T=2048, D=1024, H=16, HD=64.

Sharding: data-parallel over (batch, query-block): core c handles batch c//4
and queries [(c%4)*512, (c%4+1)*512). The head-axis softmax is local (each
core holds all 16 heads for its query slice). K/V for the whole batch are
recomputed per core (collectives are priced far above their compute saving
by the cost model, so no cross-core exchange).

Structure (all chosen against the TimelineSim cost model):
  - host feeds x^T fp16 with columns ROTATED so the core's own 512 queries
    are columns 0:512 (one SPMD program, per-core data). Key order is a
    rotation, which attention is invariant to.
  - attention runs as two passes over the 16 key chunks (qh = 256-query
    halves) to fit PSUM. Pass A also produces K/V, software-pipelined as
    per-chunk lookahead filler (2 k^T tiles + 1 v tile per chunk) emitted
    BETWEEN a chunk's scores and its PV so the PE never stalls on the
    softmax chain; PV lags one chunk.
  - PV uses the attn[q, d] orientation: lhsT = P^T tile [keys, q] (M=128),
    rhs = v [keys, 64] (N=64) -> full 128-partition output at half the PE
    cycles of the attn^T[d, q] orientation. Accumulation runs over all 16
    key chunks directly in PSUM (pre-zeroed banks + start=False: column-
    split accumulation groups must not use start=True, which clears the
    whole partition row of a bank on this HW). No spill-adds.
  - attn[q, d] -> attn^T[d, q] via dma_start_transpose (xbar): DMA cycles
    only, no PE/ACT/DVE time, no PSUM.
  - softmax: exp on ScalarE (fused scale + PSUM evacuation, [128,1024]
    pieces, double-buffered so ACT pipelines with the scores matmuls);
    head-sum tree + reciprocal on VectorE; P = E*R split VectorE/GpSimd.
  - the output projection runs at the tail (PSUM is fully booked during
    pass B); its latency chain is short.
"""

import numpy as np

import concourse.bacc as bacc
import concourse.mybir as mybir
import concourse.tile as tile
from concourse import bass_utils

B, T, D, H = 2, 2048, 1024, 16
HD = D // H          # 64
SCALE = HD ** -0.5   # 0.125
NCORES = 8
QS = B * T // NCORES  # 512 queries per core
QH = QS // 2          # 256-query halves (PSUM budget)
DC = D // 128         # 8 d/e chunks of 128
TC = T // 128         # 16 key chunks of 128

F16 = mybir.dt.float16
F32 = mybir.dt.float32
ADD = mybir.AluOpType.add
MULT = mybir.AluOpType.mult
EXP = mybir.ActivationFunctionType.Exp

_CACHED_NC = None


def _build_nc():
    nc = bacc.Bacc(
        "TRN2", target_bir_lowering=False, debug=False, enable_asserts=False
    )

    xt_d = nc.dram_tensor("xt", [D, T], F16, kind="ExternalInput").ap()
    wq_d = nc.dram_tensor("wq", [D, D], F16, kind="ExternalInput").ap()
    wk_d = nc.dram_tensor("wk", [D, D], F16, kind="ExternalInput").ap()
    wv_d = nc.dram_tensor("wv", [D, D], F16, kind="ExternalInput").ap()
    wp_d = nc.dram_tensor("wp", [D, D], F16, kind="ExternalInput").ap()
    bias_d = nc.dram_tensor("bias", [128, D], F32, kind="ExternalInput").ap()
    out_d = nc.dram_tensor("out", [QS, D], F32, kind="ExternalOutput").ap()

    def chunked(ap):  # [(c p), f] -> [p, c, f]
        return ap.rearrange("(c p) f -> p c f", p=128)

    xt_ch = chunked(xt_d)
    wq_ch = chunked(wq_d)
    out_ch = chunked(out_d)

    with tile.TileContext(nc) as tc:
        with tc.tile_pool(name="persist", bufs=1) as pp:
            kT = pp.tile([128, DC, T], F16)      # k^T: [e, t], e-chunk major
            v_sb = pp.tile([128, TC, D], F16)    # v: [t, e], t-chunk major
            # zero-padded q^T: for head pair pr and query half qh, columns
            # [0:QH] hold head 2pr's q^T at partitions 0:64 (zeros below),
            # columns [QH:2QH] hold head 2pr+1's at partitions 64:128, so
            # every scores matmul is a full-128-partition K=128 matmul.
            qpad = pp.tile([128, DC, 2, 2 * QH], F16)
            att = pp.tile([128, 2, D], F16)      # attn [q, d], per-qh reuse
            aT = pp.tile([128, DC, QS], F16)     # attn^T [d, q]
            wp_sb = pp.tile([128, DC, D], F16)
            bi_sb = pp.tile([128, D], F32)

            nc.gpsimd.memset(qpad, 0.0)

            with tc.tile_pool(name="pA", bufs=1) as pA:
                xT = pA.tile([128, DC, T], F16)
                wk_sb = pA.tile([128, DC, D], F16)
                wv_sb = pA.tile([128, DC, D], F16)

                with (
                    tc.tile_pool(name="qpool", bufs=1) as qp,
                    tc.tile_pool(name="qpsum", bufs=2, space="PSUM") as qpsum,
                ):
                    wq_sb = qp.tile([128, DC, D], F16)
                    # DMA ring order: first half of wq -> own x^T piece ->
                    # rest of wq -> wk -> remaining x^T pieces -> wv -> wp
                    # -> bias. Q gates on the first three; K production on
                    # wk + successive x^T pieces; wp/bias only matter at
                    # the tail.
                    nc.sync.dma_start(wq_sb[:, :, 0:128], wq_ch[:, :, 0:128])
                    nc.sync.dma_start(xT[:, :, 0:512], xt_ch[:, :, 0:512])
                    nc.sync.dma_start(
                        wq_sb[:, :, 128:1024], wq_ch[:, :, 128:1024]
                    )
                    nc.sync.dma_start(wk_sb, chunked(wk_d))
                    for tj in range(1, 4):
                        nc.sync.dma_start(
                            xT[:, :, tj * 512:(tj + 1) * 512],
                            xt_ch[:, :, tj * 512:(tj + 1) * 512],
                        )
                    nc.sync.dma_start(wv_sb, chunked(wv_d))
                    nc.sync.dma_start(wp_sb, chunked(wp_d))
                    nc.sync.dma_start(bi_sb, bias_d)

                    # q^T[e, q] for this core's queries (x^T cols 0:512),
                    # written into the zero-padded layout.
                    for ej in range(DC):
                        qps = qpsum.tile([128, 512], F32, tag="qps")
                        for jd in range(DC):
                            nc.tensor.matmul(
                                qps,
                                lhsT=wq_sb[:, jd, ej * 128:(ej + 1) * 128],
                                rhs=xT[:, jd, 0:512],
                                start=(jd == 0),
                                stop=(jd == DC - 1),
                            )
                        cp = (
                            nc.scalar.copy if ej % 2 == 0
                            else nc.vector.tensor_copy
                        )
                        for sel in range(2):
                            cp(
                                qpad[0:64, ej, sel, 0:QH],
                                qps[0:64, sel * QH:(sel + 1) * QH],
                            )
                            cp(
                                qpad[64:128, ej, sel, QH:2 * QH],
                                qps[64:128, sel * QH:(sel + 1) * QH],
                            )

                # ---------------- pass A: qh=0 + K/V production ----------
                with (
                    tc.tile_pool(name="accA", bufs=1, space="PSUM") as accp,
                    tc.tile_pool(name="scA", bufs=1, space="PSUM") as scp,
                    tc.tile_pool(name="kvps", bufs=2, space="PSUM") as kvp,
                    tc.tile_pool(name="Ep", bufs=2) as Ep,
                    tc.tile_pool(name="smx", bufs=2) as smx,
                    tc.tile_pool(name="rpool", bufs=2) as rpool,
                ):
                    def emit_k(tj, ej):
                        kps = kvp.tile([128, 512], F32, tag="kv")
                        for jd in range(DC):
                            nc.tensor.matmul(
                                kps,
                                lhsT=wk_sb[:, jd, ej * 128:(ej + 1) * 128],
                                rhs=xT[:, jd, tj * 512:(tj + 1) * 512],
                                start=(jd == 0),
                                stop=(jd == DC - 1),
                            )
                        nc.scalar.copy(
                            kT[:, ej, tj * 512:(tj + 1) * 512], kps
                        )

                    def emit_v(kc):
                        for eh in range(2):
                            vps = kvp.tile([128, 512], F32, tag="kv")
                            for jd in range(DC):
                                nc.tensor.matmul(
                                    vps,
                                    lhsT=xT[:, jd, kc * 128:(kc + 1) * 128],
                                    rhs=wv_sb[:, jd,
                                              eh * 512:(eh + 1) * 512],
                                    start=(jd == 0),
                                    stop=(jd == DC - 1),
                                )
                            cp = (
                                nc.vector.tensor_copy if eh == 0
                                else nc.scalar.copy
                            )
                            cp(v_sb[:, kc, eh * 512:(eh + 1) * 512], vps)

                    acc0 = accp.tile([128, D], F32)
                    acc1 = accp.tile([128, D], F32)
                    accs = [acc0, acc1]
                    # column-split accumulation groups share PSUM banks;
                    # start=True clears beyond its own columns on this HW,
                    # so pre-zero the banks and accumulate with start=False.
                    nc.vector.memset(acc0, 0.0)
                    nc.vector.memset(acc1, 0.0)

                    # prologue: k^T superstep 0 (keys 0:512) + v chunk 0
                    for ej in range(DC):
                        emit_k(0, ej)
                    emit_v(0)

                    pend = []  # softmax+PV closures, lagged one chunk
                    for kc in range(TC):
                        # K/V lookahead fillers, emitted BETWEEN score
                        # groups: the scores PSUM tile is single-buffered
                        # (bank budget), so group g+1's matmuls wait on
                        # group g's exp — the filler keeps the PE busy
                        # through that and through the softmax chain.
                        fillers = []
                        if kc < 12:
                            tj = kc // 4 + 1
                            fillers.append(
                                lambda tj=tj, e=2 * (kc % 4): emit_k(tj, e)
                            )
                            fillers.append(
                                lambda tj=tj, e=2 * (kc % 4) + 1:
                                emit_k(tj, e)
                            )
                        if kc < TC - 1:
                            fillers.append(lambda kc=kc: emit_v(kc + 1))
                        Et = _scores(nc, scp, Ep, kT, qpad, kc, qh=0,
                                     fillers=fillers)
                        pend.append(
                            lambda kc=kc, Et=Et: _softmax_pv(
                                nc, smx, rpool, v_sb, accs, Et, kc,
                                dve_heads=10
                            )
                        )
                        if len(pend) > 1:
                            pend.pop(0)()
                    pend.pop(0)()
                    nc.scalar.copy(att[:, 0, :], acc0)
                    nc.vector.tensor_copy(att[:, 1, :], acc1)

            # attn^T for qh0 via xbar DMA transpose (DMA engine only)
            for qc in range(2):
                nc.sync.dma_start_transpose(
                    aT[:, :, qc * 128:(qc + 1) * 128], att[:, qc, :]
                )

            # ---------------- pass B: qh=1 ----------
            with (
                tc.tile_pool(name="accB", bufs=1, space="PSUM") as accpB,
                tc.tile_pool(name="scB", bufs=2, space="PSUM") as scpB,
                tc.tile_pool(name="EpB", bufs=4) as EpB,
                tc.tile_pool(name="smxB", bufs=2) as smxB,
                tc.tile_pool(name="rpoolB", bufs=2) as rpoolB,
            ):
                # pass B accumulates attn^T [d, q] directly (PV orientation
                # A, 2x the PE cycles of orientation B — free here since
                # pass B is ACT-bound) so the tail needs no evac+transpose.
                accT = accpB.tile([128, DC, QH], F32)
                nc.vector.memset(accT, 0.0)
                pend = []
                for kc in range(TC):
                    Et = _scores(nc, scpB, EpB, kT, qpad, kc, qh=1)
                    pend.append(
                        lambda kc=kc, Et=Et: _softmax_pvT(
                            nc, smxB, rpoolB, v_sb, accT, Et, kc,
                            dve_heads=10
                        )
                    )
                    if len(pend) > 2:
                        pend.pop(0)()
                while pend:
                    pend.pop(0)()
                nc.scalar.copy(aT[:, 0:4, 256:512], accT[:, 0:4, :])
                nc.vector.tensor_copy(aT[:, 4:8, 256:512], accT[:, 4:8, :])

            # ---------------- tail: output projection ----------
            with (
                tc.tile_pool(name="prjps", bufs=2, space="PSUM") as prjp,
                tc.tile_pool(name="outp", bufs=2) as outp,
            ):
                # qh0's projection first: its aT half has been ready since
                # pass A, so it overlaps qh1's evac/transpose chain. Wide
                # [128,1024] tiles: the two e-halves land in separate banks
                # (start=True safe), one bias add + one DMA per block.
                for qs in (0, 1, 2, 3):
                    pm = prjp.tile([128, D], F32, tag="pm")
                    for eh in range(2):
                        for jd in range(DC):
                            nc.tensor.matmul(
                                pm[:, eh * 512:(eh + 1) * 512],
                                lhsT=aT[:, jd, qs * 128:(qs + 1) * 128],
                                rhs=wp_sb[:, jd, eh * 512:(eh + 1) * 512],
                                start=(jd == 0),
                                stop=(jd == DC - 1),
                            )
                    ot = outp.tile([128, D], F32, tag="ot")
                    nc.vector.tensor_tensor(ot, pm, bi_sb, ADD)
                    nc.sync.dma_start(out_ch[:, qs, :], ot)

    nc.compile()
    return nc


def _scores(nc, scp, Ep, kT, qpad, kc, qh, fillers=()):
    """QK^T scores + fused scale/exp evacuation for one key chunk.

    `fillers` are emitted between score groups to give the PE independent
    work while the single-buffered scores tile round-trips through exp.
    """
    fillers = list(fillers)
    Et = Ep.tile([128, H, QH], F16, tag="E")
    for g in range(4):
        sc = scp.tile([128, 1024], F32, tag="sc")
        for i in range(2):
            pr = 2 * g + i
            nc.tensor.matmul(
                sc[:, i * 512:(i + 1) * 512],
                lhsT=kT[:, pr, kc * 128:(kc + 1) * 128],
                rhs=qpad[:, pr, qh, :],
                start=True,
                stop=True,
            )
        nc.scalar.activation(Et[:, 4 * g:4 * g + 4, :], sc, EXP, scale=SCALE)
        if fillers:
            fillers.pop(0)()
    while fillers:
        fillers.pop(0)()
    return Et


def _softmax_pv(nc, smx, rpool, v_sb, accs, Et, kc, dve_heads):
    """Head-axis softmax + PV accumulation for one key chunk."""
    # S = sum over heads (log tree), R = 1/S, P = E * R broadcast
    tmp = smx.tile([128, H // 2, QH], F16, tag="tmp")
    nc.vector.tensor_tensor(tmp, Et[:, 0:8], Et[:, 8:16], ADD)
    nc.vector.tensor_tensor(tmp[:, 0:4], tmp[:, 0:4], tmp[:, 4:8], ADD)
    nc.vector.tensor_tensor(tmp[:, 0:2], tmp[:, 0:2], tmp[:, 2:4], ADD)
    nc.vector.tensor_tensor(tmp[:, 0:1], tmp[:, 0:1], tmp[:, 1:2], ADD)
    r = rpool.tile([128, 1, QH], F16, tag="r")
    with nc.allow_low_precision(
        reason="softmax denominator reciprocal in fp16"
    ):
        nc.vector.reciprocal(r, tmp[:, 0:1])
    a = dve_heads
    nc.vector.tensor_tensor(
        Et[:, 0:a], Et[:, 0:a], r.to_broadcast([128, a, QH]), MULT
    )
    nc.gpsimd.tensor_tensor(
        Et[:, a:H], Et[:, a:H], r.to_broadcast([128, H - a, QH]), MULT
    )
    # PV: attn[q, d] orientation, PSUM accumulation across all key chunks
    for h in range(H):
        for qc in range(2):
            nc.tensor.matmul(
                accs[qc][:, h * HD:(h + 1) * HD],
                lhsT=Et[:, h, qc * 128:(qc + 1) * 128],
                rhs=v_sb[:, kc, h * HD:(h + 1) * HD],
                start=False,
                stop=(kc == TC - 1),
                skip_group_check=True,
            )


def _softmax_pvT(nc, smx, rpool, v_sb, accT, Et, kc, dve_heads):
    """Like _softmax_pv, but PV writes attn^T [d, q] (orientation A):
    lhsT = v tile [keys, 64], rhs = P^T [keys, q]; head pairs share a
    PSUM region via partition halves."""
    tmp = smx.tile([128, H // 2, QH], F16, tag="tmp")
    nc.vector.tensor_tensor(tmp, Et[:, 0:8], Et[:, 8:16], ADD)
    nc.vector.tensor_tensor(tmp[:, 0:4], tmp[:, 0:4], tmp[:, 4:8], ADD)
    nc.vector.tensor_tensor(tmp[:, 0:2], tmp[:, 0:2], tmp[:, 2:4], ADD)
    nc.vector.tensor_tensor(tmp[:, 0:1], tmp[:, 0:1], tmp[:, 1:2], ADD)
    r = rpool.tile([128, 1, QH], F16, tag="r")
    with nc.allow_low_precision(
        reason="softmax denominator reciprocal in fp16"
    ):
        nc.vector.reciprocal(r, tmp[:, 0:1])
    a = dve_heads
    nc.vector.tensor_tensor(
        Et[:, 0:a], Et[:, 0:a], r.to_broadcast([128, a, QH]), MULT
    )
    nc.gpsimd.tensor_tensor(
        Et[:, a:H], Et[:, a:H], r.to_broadcast([128, H - a, QH]), MULT
    )
    for h in range(H):
        dc, half = h // 2, h % 2
        lo = 64 * half
        nc.tensor.matmul(
            accT[lo:lo + 64, dc, :],
            lhsT=v_sb[:, kc, h * HD:(h + 1) * HD],
            rhs=Et[:, h, :],
            start=False,
            stop=(kc == TC - 1),
            skip_group_check=True,
        )


def get_nc():
    global _CACHED_NC
    if _CACHED_NC is None:
        _CACHED_NC = _build_nc()
    return _CACHED_NC


def kernel(x, w_qkv, w_proj, b_proj, _trace=False, _tmpdir=None):
    x = np.asarray(x, dtype=np.float32)
    w_qkv = np.asarray(w_qkv, dtype=np.float32)
    w_proj = np.asarray(w_proj, dtype=np.float32)
    b_proj = np.asarray(b_proj, dtype=np.float32)

    # Host-side layout prep: transpose + fp16 casts + per-core rotation.
    xT = [np.ascontiguousarray(x[b].T).astype(np.float16) for b in range(B)]
    wq = np.ascontiguousarray(w_qkv[:, 0:D]).astype(np.float16)
    wk = np.ascontiguousarray(w_qkv[:, D:2 * D]).astype(np.float16)
    wv = np.ascontiguousarray(w_qkv[:, 2 * D:3 * D]).astype(np.float16)
    wp = w_proj.astype(np.float16)
    bias = np.ascontiguousarray(
        np.broadcast_to(b_proj, (128, D))
    ).astype(np.float32)

    in_maps = []
    for c in range(NCORES):
        b = c // (NCORES // B)
        qofs = (c % (NCORES // B)) * QS
        xt_rot = np.ascontiguousarray(np.roll(xT[b], -qofs, axis=1))
        in_maps.append(
            {
                "xt": xt_rot,
                "wq": wq,
                "wk": wk,
                "wv": wv,
                "wp": wp,
                "bias": bias,
            }
        )

    nc = get_nc()
    res = bass_utils.run_bass_kernel_spmd(
        nc,
        in_maps,
        core_ids=list(range(NCORES)),
        trace=_trace,
        tmpdir=_tmpdir,
    )

    out = np.empty((B, T, D), dtype=np.float32)
    for c in range(NCORES):
        b = c // (NCORES // B)
        qofs = (c % (NCORES // B)) * QS
        out[b, qofs:qofs + QS] = res.results[c]["out"]
    if _trace:
        kernel._last_results = res
    return out


# revision 39
# speedup vs baseline: 1.0208x; 1.0142x over previous
"""Trainium2 Bass kernel for nn_Attention_46995532153449.

Module: qkv = x @ w_qkv; per-head scores = q k^T * hd^-0.5; softmax over the
HEAD axis (axis=1); attn = probs @ v; out = attn @ w_proj + b_proj.

Shapes: B=2, T=2048, D=1024, H=16, HD=64.

Sharding: data-parallel over (batch, query-block): core c handles batch c//4
and queries [(c%4)*512, (c%4+1)*512). The head-axis softmax is local (each
core holds all 16 heads for its query slice). K/V for the whole batch are
recomputed per core (collectives are priced far above their compute saving
by the cost model, so no cross-core exchange).

Structure (all chosen against the TimelineSim cost model):
  - host feeds x^T fp16 with columns ROTATED so the core's own 512 queries
    are columns 0:512 (one SPMD program, per-core data). Key order is a
    rotation, which attention is invariant to.
  - attention runs as two passes over the 16 key chunks (qh = 256-query
    halves) to fit PSUM. Pass A also produces K/V, software-pipelined as
    per-chunk lookahead filler (2 k^T tiles + 1 v tile per chunk) emitted
    BETWEEN a chunk's scores and its PV so the PE never stalls on the
    softmax chain; PV lags one chunk.
  - PV uses the attn[q, d] orientation: lhsT = P^T tile [keys, q] (M=128),
    rhs = v [keys, 64] (N=64) -> full 128-partition output at half the PE
    cycles of the attn^T[d, q] orientation. Accumulation runs over all 16
    key chunks directly in PSUM (pre-zeroed banks + start=False: column-
    split accumulation groups must not use start=True, which clears the
    whole partition row of a bank on this HW). No spill-adds.
  - attn[q, d] -> attn^T[d, q] via dma_start_transpose (xbar): DMA cycles
    only, no PE/ACT/DVE time, no PSUM.
  - softmax: exp on ScalarE (fused scale + PSUM evacuation, [128,1024]
    pieces, double-buffered so ACT pipelines with the scores matmuls);
    head-sum tree + reciprocal on VectorE; P = E*R split VectorE/GpSimd.
  - the output projection runs at the tail (PSUM is fully booked during
    pass B); its latency chain is short.
"""

import numpy as np

import concourse.bacc as bacc
import concourse.mybir as mybir
import concourse.tile as tile
from concourse import bass_utils

B, T, D, H = 2, 2048, 1024, 16
HD = D // H          # 64
SCALE = HD ** -0.5   # 0.125
NCORES = 8
QS = B * T // NCORES  # 512 queries per core
QH = QS // 2          # 256-query halves (PSUM budget)
DC = D // 128         # 8 d/e chunks of 128
TC = T // 128         # 16 key chunks of 128

F16 = mybir.dt.float16
F32 = mybir.dt.float32
ADD = mybir.AluOpType.add
MULT = mybir.AluOpType.mult
EXP = mybir.ActivationFunctionType.Exp

_CACHED_NC = None


def _build_nc():
    nc = bacc.Bacc(
        "TRN2", target_bir_lowering=False, debug=False, enable_asserts=False
    )

    xt_d = nc.dram_tensor("xt", [D, T], F16, kind="ExternalInput").ap()
    wq_d = nc.dram_tensor("wq", [D, D], F16, kind="ExternalInput").ap()
    wk_d = nc.dram_tensor("wk", [D, D], F16, kind="ExternalInput").ap()
    wv_d = nc.dram_tensor("wv", [D, D], F16, kind="ExternalInput").ap()
    wp_d = nc.dram_tensor("wp", [D, D], F16, kind="ExternalInput").ap()
    bias_d = nc.dram_tensor("bias", [128, D], F32, kind="ExternalInput").ap()
    out_d = nc.dram_tensor("out", [QS, D], F32, kind="ExternalOutput").ap()

    def chunked(ap):  # [(c p), f] -> [p, c, f]
        return ap.rearrange("(c p) f -> p c f", p=128)

    xt_ch = chunked(xt_d)
    wq_ch = chunked(wq_d)
    out_ch = chunked(out_d)

    with tile.TileContext(nc) as tc:
        with tc.tile_pool(name="persist", bufs=1) as pp:
            kT = pp.tile([128, DC, T], F16)      # k^T: [e, t], e-chunk major
            v_sb = pp.tile([128, TC, D], F16)    # v: [t, e], t-chunk major
            # zero-padded q^T: for head pair pr and query half qh, columns
            # [0:QH] hold head 2pr's q^T at partitions 0:64 (zeros below),
            # columns [QH:2QH] hold head 2pr+1's at partitions 64:128, so
            # every scores matmul is a full-128-partition K=128 matmul.
            qpad = pp.tile([128, DC, 2, 2 * QH], F16)
            att = pp.tile([128, 2, D], F16)      # attn [q, d], per-qh reuse
            aT = pp.tile([128, DC, QS], F16)     # attn^T [d, q]
            wp_sb = pp.tile([128, DC, D], F16)
            bi_sb = pp.tile([128, D], F32)

            nc.gpsimd.memset(qpad, 0.0)

            with tc.tile_pool(name="pA", bufs=1) as pA:
                xT = pA.tile([128, DC, T], F16)
                wk_sb = pA.tile([128, DC, D], F16)
                wv_sb = pA.tile([128, DC, D], F16)

                with (
                    tc.tile_pool(name="qpool", bufs=1) as qp,
                    tc.tile_pool(name="qpsum", bufs=2, space="PSUM") as qpsum,
                ):
                    wq_sb = qp.tile([128, DC, D], F16)
                    # DMA ring order: first half of wq -> own x^T piece ->
                    # rest of wq -> wk -> remaining x^T pieces -> wv -> wp
                    # -> bias. Q gates on the first three; K production on
                    # wk + successive x^T pieces; wp/bias only matter at
                    # the tail.
                    nc.sync.dma_start(wq_sb[:, :, 0:128], wq_ch[:, :, 0:128])
                    nc.sync.dma_start(xT[:, :, 0:512], xt_ch[:, :, 0:512])
                    nc.sync.dma_start(
                        wq_sb[:, :, 128:1024], wq_ch[:, :, 128:1024]
                    )
                    nc.sync.dma_start(wk_sb, chunked(wk_d))
                    for tj in range(1, 4):
                        nc.sync.dma_start(
                            xT[:, :, tj * 512:(tj + 1) * 512],
                            xt_ch[:, :, tj * 512:(tj + 1) * 512],
                        )
                    nc.sync.dma_start(wv_sb, chunked(wv_d))
                    nc.sync.dma_start(wp_sb, chunked(wp_d))
                    nc.sync.dma_start(bi_sb, bias_d)

                    # q^T[e, q] for this core's queries (x^T cols 0:512),
                    # written into the zero-padded layout.
                    for ej in range(DC):
                        qps = qpsum.tile([128, 512], F32, tag="qps")
                        for jd in range(DC):
                            nc.tensor.matmul(
                                qps,
                                lhsT=wq_sb[:, jd, ej * 128:(ej + 1) * 128],
                                rhs=xT[:, jd, 0:512],
                                start=(jd == 0),
                                stop=(jd == DC - 1),
                            )
                        cp = (
                            nc.scalar.copy if ej % 2 == 0
                            else nc.vector.tensor_copy
                        )
                        for sel in range(2):
                            cp(
                                qpad[0:64, ej, sel, 0:QH],
                                qps[0:64, sel * QH:(sel + 1) * QH],
                            )
                            cp(
                                qpad[64:128, ej, sel, QH:2 * QH],
                                qps[64:128, sel * QH:(sel + 1) * QH],
                            )

                # ---------------- pass A: qh=0 + K/V production ----------
                with (
                    tc.tile_pool(name="accA", bufs=1, space="PSUM") as accp,
                    tc.tile_pool(name="scA", bufs=1, space="PSUM") as scp,
                    tc.tile_pool(name="kvps", bufs=2, space="PSUM") as kvp,
                    tc.tile_pool(name="Ep", bufs=2) as Ep,
                    tc.tile_pool(name="smx", bufs=2) as smx,
                ):
                    def emit_k(tj, ej):
                        kps = kvp.tile([128, 512], F32, tag="kv")
                        for jd in range(DC):
                            nc.tensor.matmul(
                                kps,
                                lhsT=wk_sb[:, jd, ej * 128:(ej + 1) * 128],
                                rhs=xT[:, jd, tj * 512:(tj + 1) * 512],
                                start=(jd == 0),
                                stop=(jd == DC - 1),
                            )
                        nc.scalar.copy(
                            kT[:, ej, tj * 512:(tj + 1) * 512], kps
                        )

                    def emit_v(kc):
                        for eh in range(2):
                            vps = kvp.tile([128, 512], F32, tag="kv")
                            for jd in range(DC):
                                nc.tensor.matmul(
                                    vps,
                                    lhsT=xT[:, jd, kc * 128:(kc + 1) * 128],
                                    rhs=wv_sb[:, jd,
                                              eh * 512:(eh + 1) * 512],
                                    start=(jd == 0),
                                    stop=(jd == DC - 1),
                                )
                            cp = (
                                nc.vector.tensor_copy if eh == 0
                                else nc.scalar.copy
                            )
                            cp(v_sb[:, kc, eh * 512:(eh + 1) * 512], vps)

                    acc0 = accp.tile([128, D], F32)
                    acc1 = accp.tile([128, D], F32)
                    accs = [acc0, acc1]
                    # column-split accumulation groups share PSUM banks;
                    # start=True clears beyond its own columns on this HW,
                    # so pre-zero the banks and accumulate with start=False.
                    nc.vector.memset(acc0, 0.0)
                    nc.vector.memset(acc1, 0.0)

                    # prologue: k^T superstep 0 (keys 0:512) + v chunk 0
                    for ej in range(DC):
                        emit_k(0, ej)
                    emit_v(0)

                    pend = []  # softmax+PV closures, lagged one chunk
                    for kc in range(TC):
                        # K/V lookahead fillers, emitted BETWEEN score
                        # groups: the scores PSUM tile is single-buffered
                        # (bank budget), so group g+1's matmuls wait on
                        # group g's exp — the filler keeps the PE busy
                        # through that and through the softmax chain.
                        fillers = []
                        if kc < 12:
                            tj = kc // 4 + 1
                            fillers.append(
                                lambda tj=tj, e=2 * (kc % 4): emit_k(tj, e)
                            )
                            fillers.append(
                                lambda tj=tj, e=2 * (kc % 4) + 1:
                                emit_k(tj, e)
                            )
                        if kc < TC - 1:
                            fillers.append(lambda kc=kc: emit_v(kc + 1))
                        Et = _scores(nc, scp, Ep, kT, qpad, kc, qh=0,
                                     fillers=fillers)
                        pend.append(
                            lambda kc=kc, Et=Et: _softmax_pv(
                                nc, smx, v_sb, accs, Et, kc, dve_heads=10
                            )
                        )
                        if len(pend) > 1:
                            pend.pop(0)()
                    pend.pop(0)()
                    nc.scalar.copy(att[:, 0, :], acc0)
                    nc.vector.tensor_copy(att[:, 1, :], acc1)

            # attn^T for qh0 via xbar DMA transpose (DMA engine only)
            for qc in range(2):
                nc.sync.dma_start_transpose(
                    aT[:, :, qc * 128:(qc + 1) * 128], att[:, qc, :]
                )

            # ---------------- pass B: qh=1 ----------
            with (
                tc.tile_pool(name="accB", bufs=1, space="PSUM") as accpB,
                tc.tile_pool(name="scB", bufs=2, space="PSUM") as scpB,
                tc.tile_pool(name="EpB", bufs=4) as EpB,
                tc.tile_pool(name="smxB", bufs=2) as smxB,
            ):
                accB0 = accpB.tile([128, D], F32)
                accB1 = accpB.tile([128, D], F32)
                accsB = [accB0, accB1]
                nc.vector.memset(accB0, 0.0)
                nc.vector.memset(accB1, 0.0)
                pend = []  # PV lags two chunks: covers the softmax latency
                for kc in range(TC):
                    Et = _scores(nc, scpB, EpB, kT, qpad, kc, qh=1)
                    pend.append(
                        lambda kc=kc, Et=Et: _softmax_pv(
                            nc, smxB, v_sb, accsB, Et, kc, dve_heads=10
                        )
                    )
                    if len(pend) > 2:
                        pend.pop(0)()
                while pend:
                    pend.pop(0)()
                nc.scalar.copy(att[:, 0, :], accB0)
                nc.vector.tensor_copy(att[:, 1, :], accB1)
            for qc in range(2):
                nc.sync.dma_start_transpose(
                    aT[:, :, 256 + qc * 128:256 + (qc + 1) * 128],
                    att[:, qc, :],
                )

            # ---------------- tail: output projection ----------
            with (
                tc.tile_pool(name="prjps", bufs=4, space="PSUM") as prjp,
                tc.tile_pool(name="outp", bufs=4) as outp,
            ):
                # qh0's projection first: its aT half has been ready since
                # pass A, so it overlaps qh1's evac/transpose chain.
                for qs in (0, 1, 2, 3):
                    for eh in range(2):
                        pm = prjp.tile([128, 512], F32, tag="pm")
                        for jd in range(DC):
                            nc.tensor.matmul(
                                pm,
                                lhsT=aT[:, jd, qs * 128:(qs + 1) * 128],
                                rhs=wp_sb[:, jd, eh * 512:(eh + 1) * 512],
                                start=(jd == 0),
                                stop=(jd == DC - 1),
                            )
                        ot = outp.tile([128, 512], F32, tag="ot")
                        nc.vector.tensor_tensor(
                            ot, pm, bi_sb[:, eh * 512:(eh + 1) * 512], ADD
                        )
                        nc.sync.dma_start(
                            out_ch[:, qs, eh * 512:(eh + 1) * 512], ot
                        )

    nc.compile()
    return nc


def _scores(nc, scp, Ep, kT, qpad, kc, qh, fillers=()):
    """QK^T scores + fused scale/exp evacuation for one key chunk.

    `fillers` are emitted between score groups to give the PE independent
    work while the single-buffered scores tile round-trips through exp.
    """
    fillers = list(fillers)
    Et = Ep.tile([128, H, QH], F16, tag="E")
    for g in range(4):
        sc = scp.tile([128, 1024], F32, tag="sc")
        for i in range(2):
            pr = 2 * g + i
            nc.tensor.matmul(
                sc[:, i * 512:(i + 1) * 512],
                lhsT=kT[:, pr, kc * 128:(kc + 1) * 128],
                rhs=qpad[:, pr, qh, :],
                start=True,
                stop=True,
            )
        nc.scalar.activation(Et[:, 4 * g:4 * g + 4, :], sc, EXP, scale=SCALE)
        if fillers:
            fillers.pop(0)()
    while fillers:
        fillers.pop(0)()
    return Et


def _softmax_pv(nc, smx, v_sb, accs, Et, kc, dve_heads):
    """Head-axis softmax + PV accumulation for one key chunk."""
    # S = sum over heads (log tree), R = 1/S, P = E * R broadcast
    tmp = smx.tile([128, H // 2, QH], F16, tag="tmp")
    nc.vector.tensor_tensor(tmp, Et[:, 0:8], Et[:, 8:16], ADD)
    nc.vector.tensor_tensor(tmp[:, 0:4], tmp[:, 0:4], tmp[:, 4:8], ADD)
    nc.vector.tensor_tensor(tmp[:, 0:2], tmp[:, 0:2], tmp[:, 2:4], ADD)
    nc.vector.tensor_tensor(tmp[:, 0:1], tmp[:, 0:1], tmp[:, 1:2], ADD)
    r = smx.tile([128, 1, QH], F16, tag="r")
    with nc.allow_low_precision(
        reason="softmax denominator reciprocal in fp16"
    ):
        nc.vector.reciprocal(r, tmp[:, 0:1])
    a = dve_heads
    nc.vector.tensor_tensor(
        Et[:, 0:a], Et[:, 0:a], r.to_broadcast([128, a, QH]), MULT
    )
    nc.gpsimd.tensor_tensor(
        Et[:, a:H], Et[:, a:H], r.to_broadcast([128, H - a, QH]), MULT
    )
    # PV: attn[q, d] orientation, PSUM accumulation across all key chunks
    for h in range(H):
        for qc in range(2):
            nc.tensor.matmul(
                accs[qc][:, h * HD:(h + 1) * HD],
                lhsT=Et[:, h, qc * 128:(qc + 1) * 128],
                rhs=v_sb[:, kc, h * HD:(h + 1) * HD],
                start=False,
                stop=(kc == TC - 1),
                skip_group_check=True,
            )


def get_nc():
    global _CACHED_NC
    if _CACHED_NC is None:
        _CACHED_NC = _build_nc()
    return _CACHED_NC


def kernel(x, w_qkv, w_proj, b_proj, _trace=False, _tmpdir=None):
    x = np.asarray(x, dtype=np.float32)
    w_qkv = np.asarray(w_qkv, dtype=np.float32)
    w_proj = np.asarray(w_proj, dtype=np.float32)
    b_proj = np.asarray(b_proj, dtype=np.float32)

    # Host-side layout prep: transpose + fp16 casts + per-core rotation.
    xT = [np.ascontiguousarray(x[b].T).astype(np.float16) for b in range(B)]
    wq = np.ascontiguousarray(w_qkv[:, 0:D]).astype(np.float16)
    wk = np.ascontiguousarray(w_qkv[:, D:2 * D]).astype(np.float16)
    wv = np.ascontiguousarray(w_qkv[:, 2 * D:3 * D]).astype(np.float16)
    wp = w_proj.astype(np.float16)
    bias = np.ascontiguousarray(
        np.broadcast_to(b_proj, (128, D))
    ).astype(np.float32)

    in_maps = []
    for c in range(NCORES):
        b = c // (NCORES // B)
        qofs = (c % (NCORES // B)) * QS
        xt_rot = np.ascontiguousarray(np.roll(xT[b], -qofs, axis=1))
        in_maps.append(
            {
                "xt": xt_rot,
                "wq": wq,
                "wk": wk,
                "wv": wv,
                "wp": wp,
                "bias": bias,
            }
        )

    nc = get_nc()
    res = bass_utils.run_bass_kernel_spmd(
        nc,
        in_maps,
        core_ids=list(range(NCORES)),
        trace=_trace,
        tmpdir=_tmpdir,
    )

    out = np.empty((B, T, D), dtype=np.float32)
    for c in range(NCORES):
        b = c // (NCORES // B)
        qofs = (c % (NCORES // B)) * QS
        out[b, qofs:qofs + QS] = res.results[c]["out"]
    if _trace:
        kernel._last_results = res
    return out
